# revision 30
# baseline (speedup 1.0000x reference)
"""Fused attention layer (projections + masked softmax + context) on 8 TRN2
NeuronCores, data-parallel over batch (one batch element per core).

Per core (batch b):
  pq = q @ Wq.T + bq ; pk = k @ Wk.T + bk ; pv = v @ Wv.T + bv
  scores = pq @ pk.T / 32 ;  E = exp(scores) * mask
  attn = E / rowsum(E) ;  ctx = attn @ pv

Matmuls contract over the SBUF partition dim, so activations/weights are
transposed on-chip via TensorE (identity matmul) in bf16; the f32 -> bf16
cast rides the SWDGE load DMAs for free. All matmul operands are bf16
(full PE rate; FWL weight loads); PSUM accumulation and the softmax are
f32. pq^T, pk^T and pv are all bf16 and stay fully resident in SBUF
across phase 2 (the projection epilogues write them in place; no DRAM
round trips). Phase 2 is software-pipelined: scores of panel p+1 run on
TensorE while softmax of panel p runs on ScalarE/VectorE, and the 1/den
normalization of ctx is folded into the PSUM->SBUF copyback scale.
"""
import os
from contextlib import ExitStack

import numpy as np

import concourse.bass as bass
import concourse.tile as tile
from concourse import bacc, mybir, masks
from concourse.bass_utils import run_bass_kernel_spmd

F32 = mybir.dt.float32
BF16 = mybir.dt.bfloat16
I32 = mybir.dt.int32
AF = mybir.ActivationFunctionType
ALU = mybir.AluOpType

P = 128
B = 8
QN = 2048
KN = 2048
D = 1024           # DIN == DPROJ
DC = D // P        # 8 contraction chunks
NPANEL = QN // P   # 16 query panels
NS = KN // P       # 16 key chunks
SCALE = 1.0 / 32.0 # 1/sqrt(DPROJ)

N_CORES = 8

_cached_nc = None


def _build():
    nc = bacc.Bacc("TRN2", target_bir_lowering=False, debug=False,
                   num_devices=N_CORES)

    q_d = nc.dram_tensor("query", [QN, D], F32, kind="ExternalInput").ap()
    k_d = nc.dram_tensor("key", [KN, D], F32, kind="ExternalInput").ap()
    v_d = nc.dram_tensor("value", [KN, D], F32, kind="ExternalInput").ap()
    m_d = nc.dram_tensor("mask", [1, KN], I32, kind="ExternalInput").ap()
    w_q = nc.dram_tensor("Wq", [D, D], F32, kind="ExternalInput").ap()
    b_q = nc.dram_tensor("bq", [1, D], F32, kind="ExternalInput").ap()
    w_k = nc.dram_tensor("Wk", [D, D], F32, kind="ExternalInput").ap()
    b_k = nc.dram_tensor("bk", [1, D], F32, kind="ExternalInput").ap()
    w_v = nc.dram_tensor("Wv", [D, D], F32, kind="ExternalInput").ap()
    b_v = nc.dram_tensor("bv", [1, D], F32, kind="ExternalInput").ap()

    ctx_d = nc.dram_tensor("ctx", [QN, D], F32, kind="ExternalOutput").ap()
    attn_d = nc.dram_tensor("attn", [QN, KN], F32, kind="ExternalOutput").ap()


    cb_toggle = [0]  # alternate ACT/DVE for PSUM->SBUF copybacks

    def copyback(dst, src):
        if cb_toggle[0] % 2 == 0:
            nc.scalar.activation(dst, src, AF.Copy)
        else:
            nc.vector.tensor_copy(dst, src)
        cb_toggle[0] += 1

    with tile.TileContext(nc, pool_alloc_mode="queue") as tc, ExitStack() as top:
        const_pool = top.enter_context(tc.tile_pool(name="consts", bufs=1))
        # ident_bf tile is created here but its instructions are emitted
        # after the first load DMA trigger so the GpSimd ring starts on DMAs
        ident_bf = const_pool.tile([P, P], BF16)
        zero_bias = const_pool.tile([P, 1], F32)
        nc.vector.memset(zero_bias[:], 0.0)
        bias_q = const_pool.tile([P, DC], F32)
        bias_k = const_pool.tile([P, DC], F32)

        ph1 = top.enter_context(ExitStack())
        xnat_pool = ph1.enter_context(tc.tile_pool(name="xnat", bufs=3))
        wnat_pool = ph1.enter_context(tc.tile_pool(name="wnat", bufs=2))

        def transpose_panel(x_dram, xT, s4, ps_tr):
            """Transpose 512 rows of x (f32, cast to bf16 by the DMA) into
            xT[:, :, s4*512:(s4+1)*512]."""
            xnat = xnat_pool.tile([P, 4, D], BF16, tag="xnat")
            nc.gpsimd.dma_start(
                out=xnat[:],
                in_=x_dram[s4 * 512:(s4 + 1) * 512, :]
                    .rearrange("(s p) c -> p s c", p=P))
            for cc2 in range(DC // 2):
                pt = ps_tr.tile([P, 8, P], BF16, tag="pt")
                for m in range(2):
                    cc = 2 * cc2 + m
                    for j in range(4):
                        nc.tensor.transpose(
                            pt[:, 4 * m + j, :],
                            xnat[:, j, cc * P:(cc + 1) * P], ident_bf[:])
                copyback(xT[:, 2 * cc2:2 * cc2 + 2, s4 * 512:(s4 + 1) * 512],
                         pt[:, :, :].rearrange("p (m j) f -> p m (j f)", m=2))

        def weights_T(w_dram, wt, ps_tr):
            """W [d, c] f32 natural -(cast DMA)-> bf16 -> wt [c_in_cc, cc, d]."""
            w_nat = wnat_pool.tile([P, DC, D], BF16, tag="wnat")
            nc.gpsimd.dma_start(
                out=w_nat[:],
                in_=w_dram.rearrange("(dc p) c -> p dc c", p=P))
            for cc in range(DC):
                pt = ps_tr.tile([P, 8, P], BF16, tag="pt")
                for dc in range(DC):
                    nc.tensor.transpose(
                        pt[:, dc, :],
                        w_nat[:, dc, cc * P:(cc + 1) * P], ident_bf[:])
                copyback(wt[:, cc, :], pt[:, :, :])

        # pqT, pv and pkt live through phase 2 (right-side pool stack)
        ph2 = top.enter_context(ExitStack())
        pqt_pool = ph2.enter_context(tc.tile_pool(name="pqt", bufs=1, side="right"))
        pv_pool = ph2.enter_context(tc.tile_pool(name="pv", bufs=1, side="right"))
        pkt_pool = ph2.enter_context(tc.tile_pool(name="pkt", bufs=1, side="right"))

        # ---- q step: pq^T resident (bf16) ----
        with ExitStack() as ph:
            ps_tr = ph.enter_context(tc.tile_pool(name="ps_tr_q", bufs=2, space="PSUM"))
            ps_pr = ph.enter_context(tc.tile_pool(name="ps_pr_q", bufs=3, space="PSUM"))
            wtp = ph.enter_context(tc.tile_pool(name="wt_q", bufs=1))
            wt = wtp.tile([P, DC, D], BF16)
            # first load DMA on the ring, then the identity build overlaps it
            w_nat_q = wnat_pool.tile([P, DC, D], BF16, tag="wnat")
            nc.gpsimd.dma_start(
                out=w_nat_q[:], in_=w_q.rearrange("(dc p) c -> p dc c", p=P))
            masks.make_identity(nc, ident_bf[:])
            for cc in range(DC):
                pt = ps_tr.tile([P, 8, P], BF16, tag="pt")
                for dc in range(DC):
                    nc.tensor.transpose(
                        pt[:, dc, :],
                        w_nat_q[:, dc, cc * P:(cc + 1) * P], ident_bf[:])
                copyback(wt[:, cc, :], pt[:, :, :])
            nc.gpsimd.dma_start(out=bias_q[:], in_=bass.AP(
                tensor=b_q.tensor, offset=b_q.offset, ap=[[1, P], [P, DC]]))
            nc.gpsimd.dma_start(out=bias_k[:], in_=bass.AP(
                tensor=b_k.tensor, offset=b_k.offset, ap=[[1, P], [P, DC]]))
            xtp = ph.enter_context(tc.tile_pool(name="xt_q", bufs=1))
            xT = xtp.tile([P, DC, QN], BF16)
            pqt = pqt_pool.tile([P, DC, QN], BF16)
            transpose_panel(q_d, xT, 0, ps_tr)
            for half in range(2):
                transpose_panel(q_d, xT, 2 * half + 1, ps_tr)
                if half == 0:
                    transpose_panel(q_d, xT, 2, ps_tr)
                for dc in range(DC):
                    ps_a = ps_pr.tile([P, 2, 512], F32, tag="ps_proj")
                    for cc in range(DC):
                        for n2 in range(2):
                            nc.tensor.matmul(
                                ps_a[:, n2, :],
                                lhsT=wt[:, cc, dc * P:(dc + 1) * P],
                                rhs=xT[:, cc, (2 * half + n2) * 512:(2 * half + n2 + 1) * 512],
                                start=(cc == 0), stop=(cc == DC - 1))
                    for n2 in range(2):
                        nc.scalar.activation(
                            pqt[:, dc, (2 * half + n2) * 512:(2 * half + n2 + 1) * 512],
                            ps_a[:, n2, :], AF.Identity,
                            bias=bias_q[:, dc:dc + 1], scale=1.0)

        # ---- v step: pv [k_in_s, s, d] bf16, resident ----
        with ExitStack() as ph:
            bvp = ph.enter_context(tc.tile_pool(name="bv", bufs=1))
            bv_bc = bvp.tile([P, D], F32)
            nc.gpsimd.dma_start(out=bv_bc[:], in_=b_v.to_broadcast([P, D]))
            ps_tr = ph.enter_context(tc.tile_pool(name="ps_tr_v", bufs=2, space="PSUM"))
            ps_pr = ph.enter_context(tc.tile_pool(name="ps_pr_v", bufs=3, space="PSUM"))
            wtp = ph.enter_context(tc.tile_pool(name="wt_v", bufs=1))
            wvt = wtp.tile([P, DC, D], BF16)
            weights_T(w_v, wvt, ps_tr)
            xtp = ph.enter_context(tc.tile_pool(name="vt", bufs=1))
            vT = xtp.tile([P, DC, KN], BF16)

            pv = pv_pool.tile([P, NS, D], BF16)
            for s4 in range(4):
                transpose_panel(v_d, vT, s4, ps_tr)
                for s in range(4 * s4, 4 * s4 + 4):
                    ps_v = ps_pr.tile([P, 2, 512], F32, tag="ps_proj")
                    for cc in range(DC):
                        for d2 in range(2):
                            nc.tensor.matmul(
                                ps_v[:, d2, :],
                                lhsT=vT[:, cc, s * P:(s + 1) * P],
                                rhs=wvt[:, cc, d2 * 512:(d2 + 1) * 512],
                                start=(cc == 0), stop=(cc == DC - 1))
                    for d2 in range(2):
                        nc.vector.scalar_tensor_tensor(
                            out=pv[:, s, d2 * 512:(d2 + 1) * 512],
                            in0=ps_v[:, d2, :], scalar=1.0,
                            in1=bv_bc[:, d2 * 512:(d2 + 1) * 512],
                            op0=ALU.mult, op1=ALU.add)

        # ---- k step: pk^T [d_in_dc, dc, n] bf16, resident ----
        with ExitStack() as ph:
            ps_tr = ph.enter_context(tc.tile_pool(name="ps_tr_k", bufs=2, space="PSUM"))
            ps_pr = ph.enter_context(tc.tile_pool(name="ps_pr_k", bufs=3, space="PSUM"))
            wtp = ph.enter_context(tc.tile_pool(name="wt_k", bufs=1))
            wkt = wtp.tile([P, DC, D], BF16)
            weights_T(w_k, wkt, ps_tr)
            xtp = ph.enter_context(tc.tile_pool(name="kt", bufs=1))
            kT = xtp.tile([P, DC, KN], BF16)

            pkt = pkt_pool.tile([P, DC, KN], BF16)
            transpose_panel(k_d, kT, 0, ps_tr)
            for half in range(2):
                transpose_panel(k_d, kT, 2 * half + 1, ps_tr)
                if half == 0:
                    transpose_panel(k_d, kT, 2, ps_tr)
                for dc in range(DC):
                    ps_a = ps_pr.tile([P, 2, 512], F32, tag="ps_proj")
                    for cc in range(DC):
                        for n2 in range(2):
                            nc.tensor.matmul(
                                ps_a[:, n2, :],
                                lhsT=wkt[:, cc, dc * P:(dc + 1) * P],
                                rhs=kT[:, cc, (2 * half + n2) * 512:(2 * half + n2 + 1) * 512],
                                start=(cc == 0), stop=(cc == DC - 1))
                    for n2 in range(2):
                        nc.scalar.activation(
                            pkt[:, dc, (2 * half + n2) * 512:(2 * half + n2 + 1) * 512],
                            ps_a[:, n2, :], AF.Identity,
                            bias=bias_k[:, dc:dc + 1], scale=1.0)

        ph1.close()

        # ---------------- Phase 2: attention, per query panel ----------------
        with ExitStack() as ph:
            maskp = ph.enter_context(tc.tile_pool(name="maskp", bufs=1))
            mask_f = maskp.tile([P, KN], BF16)
            with tc.tile_pool(name="mtmp", bufs=1) as mtmp:
                mask_i = mtmp.tile([P, KN], I32)
                nc.gpsimd.dma_start(out=mask_i[:], in_=m_d.to_broadcast([P, KN]))
                nc.vector.tensor_copy(mask_f[:], mask_i[:])

            e_pool = ph.enter_context(tc.tile_pool(name="em", bufs=2))
            attn_pool = ph.enter_context(tc.tile_pool(name="attnsb", bufs=2))
            at_pool = ph.enter_context(tc.tile_pool(name="attnT", bufs=2))
            ctx_pool = ph.enter_context(tc.tile_pool(name="ctxsb", bufs=2))
            small = ph.enter_context(tc.tile_pool(name="small", bufs=4))
            ps_s = ph.enter_context(tc.tile_pool(name="ps_s", bufs=2, space="PSUM"))
            ps_t = ph.enter_context(tc.tile_pool(name="ps_t", bufs=2, space="PSUM"))
            ps_c = ph.enter_context(tc.tile_pool(name="ps_c", bufs=1, space="PSUM"))

            def scores_half(p, h):
                ps = ps_s.tile([P, 2, 512], F32)
                for dc in range(DC):
                    for n2 in range(2):
                        nc.tensor.matmul(
                            ps[:, n2, :],
                            lhsT=pqt[:, dc, p * P:(p + 1) * P],
                            rhs=pkt[:, dc, (2 * h + n2) * 512:(2 * h + n2 + 1) * 512],
                            start=(dc == 0), stop=(dc == DC - 1))
                return ps

            s_half = [scores_half(0, 0), scores_half(0, 1)]

            for p in range(NPANEL):
                # softmax for panel p (ACT/DVE; overlaps next scores on PE)
                em = e_pool.tile([P, KN], BF16)
                for h in range(2):
                    nc.scalar.activation(
                        em[:, h * 1024:(h + 1) * 1024],
                        s_half[h][:, :, :], AF.Exp,
                        bias=zero_bias[:], scale=SCALE)
                den = small.tile([P, 1], F32)
                nc.vector.scalar_tensor_tensor(
                    out=em[:], in0=em[:], scalar=1.0, in1=mask_f[:],
                    op0=ALU.mult, op1=ALU.mult, accum_out=den[:])
                rden = small.tile([P, 1], F32)
                nc.vector.reciprocal(rden[:], den[:])

                if p + 1 < NPANEL:
                    ns0 = scores_half(p + 1, 0)

                # attn output row-panel (f32)
                attn_sb = attn_pool.tile([P, KN], F32)
                nc.vector.tensor_scalar_mul(attn_sb[:], em[:], rden[:])
                nc.scalar.dma_start(
                    out=attn_d[p * P:(p + 1) * P, :], in_=attn_sb[:])

                # transpose Em (bf16) -> attnT [k_in_s, s, nq]; the 1/den
                # normalization is folded into the ctx copyback scale
                attnT = at_pool.tile([P, NS, P], BF16)
                for g in range(2):
                    pt = ps_t.tile([P, 8, P], BF16)
                    for j in range(8):
                        s_idx = g * 8 + j
                        nc.tensor.transpose(
                            pt[:, j, :],
                            em[:, s_idx * P:(s_idx + 1) * P], ident_bf[:])
                    copyback(attnT[:, g * 8:(g + 1) * 8, :], pt[:, :, :])

                # ctx panel
                ps_ctx = ps_c.tile([P, 2, 512], F32)
                for s in range(NS):
                    for d2 in range(2):
                        nc.tensor.matmul(
                            ps_ctx[:, d2, :],
                            lhsT=attnT[:, s, :],
                            rhs=pv[:, s, d2 * 512:(d2 + 1) * 512],
                            start=(s == 0), stop=(s == NS - 1))
                ctx_sb = ctx_pool.tile([P, D], F32)
                nc.scalar.activation(ctx_sb[:], ps_ctx[:, :, :], AF.Copy,
                                     scale=rden[:])
                nc.scalar.dma_start(
                    out=ctx_d[p * P:(p + 1) * P, :], in_=ctx_sb[:])

                if p + 1 < NPANEL:
                    ns1 = scores_half(p + 1, 1)
                    s_half = [ns0, ns1]

    nc.compile()
    return nc


def _get_nc():
    global _cached_nc
    if _cached_nc is None:
        _cached_nc = _build()
    return _cached_nc


last_exec_time_ns = None


def kernel(**inputs):
    global last_exec_time_ns
    nc = _get_nc()
    query = np.asarray(inputs["query"], dtype=np.float32)
    key = np.asarray(inputs["key"], dtype=np.float32)
    value = np.asarray(inputs["value"], dtype=np.float32)
    mask = np.asarray(inputs["mask"], dtype=np.int32)
    Wq = np.ascontiguousarray(np.asarray(inputs["Wq"], dtype=np.float32))
    bq = np.asarray(inputs["bq"], dtype=np.float32).reshape(1, D)
    Wk = np.ascontiguousarray(np.asarray(inputs["Wk"], dtype=np.float32))
    bk = np.asarray(inputs["bk"], dtype=np.float32).reshape(1, D)
    Wv = np.ascontiguousarray(np.asarray(inputs["Wv"], dtype=np.float32))
    bv = np.asarray(inputs["bv"], dtype=np.float32).reshape(1, D)

    in_maps = []
    for b in range(B):
        in_maps.append({
            "query": np.ascontiguousarray(query[b]),
            "key": np.ascontiguousarray(key[b]),
            "value": np.ascontiguousarray(value[b]),
            "mask": np.ascontiguousarray(mask[b].reshape(1, KN)),
            "Wq": Wq, "bq": bq, "Wk": Wk, "bk": bk, "Wv": Wv, "bv": bv,
        })

    trace = bool(os.environ.get("ATTN_TRACE"))
    res = None
    last_err = None
    for attempt in range(3):
        try:
            res = run_bass_kernel_spmd(nc, in_maps,
                                       core_ids=list(range(N_CORES)),
                                       trace=trace and attempt == 0)
            break
        except Exception as e:  # transient NRT device errors: retry
            last_err = e
    if res is None:
        raise last_err
    last_exec_time_ns = res.exec_time_ns

    ctx = np.stack([res.results[b]["ctx"] for b in range(B)])
    attn = np.stack([res.results[b]["attn"] for b in range(B)])
    return (ctx, attn)


# revision 31
# speedup vs baseline: 1.1824x; 1.1824x over previous
"""Fused attention layer (projections + masked softmax + context) on 8 TRN2
NeuronCores, data-parallel over batch (one batch element per core).

Per core (batch b):
  pq = q @ Wq.T + bq ; pk = k @ Wk.T + bk ; pv = v @ Wv.T + bv
  scores = pq @ pk.T / 32 ;  E = exp(scores) * mask
  attn = E / rowsum(E) ;  ctx = attn @ pv

Matmuls contract over the SBUF partition dim, so activations/weights are
transposed on-chip via TensorE (identity matmul) in bf16; the f32 -> bf16
cast rides the SWDGE load DMAs for free. All matmul operands are bf16
(full PE rate; FWL weight loads); PSUM accumulation and the softmax are
f32. pq^T, pk^T and pv are all bf16 and stay fully resident in SBUF
across phase 2 (the projection epilogues write them in place; no DRAM
round trips). Phase 2 is software-pipelined: scores of panel p+1 run on
TensorE while softmax of panel p runs on ScalarE/VectorE, and the 1/den
normalization of ctx is folded into the PSUM->SBUF copyback scale.
"""
import os
from contextlib import ExitStack

import numpy as np

import concourse.bass as bass
import concourse.tile as tile
from concourse import bacc, mybir, masks
from concourse.bass_utils import run_bass_kernel_spmd

F32 = mybir.dt.float32
BF16 = mybir.dt.bfloat16
I32 = mybir.dt.int32
AF = mybir.ActivationFunctionType
ALU = mybir.AluOpType

P = 128
B = 8
QN = 2048
KN = 2048
D = 1024           # DIN == DPROJ
DC = D // P        # 8 contraction chunks
NPANEL = QN // P   # 16 query panels
NS = KN // P       # 16 key chunks
SCALE = 1.0 / 32.0 # 1/sqrt(DPROJ)

N_CORES = 8

_cached_nc = None


def _build():
    nc = bacc.Bacc("TRN2", target_bir_lowering=False, debug=False,
                   num_devices=N_CORES)

    q_d = nc.dram_tensor("query", [QN, D], F32, kind="ExternalInput").ap()
    k_d = nc.dram_tensor("key", [KN, D], F32, kind="ExternalInput").ap()
    v_d = nc.dram_tensor("value", [KN, D], F32, kind="ExternalInput").ap()
    m_d = nc.dram_tensor("mask", [1, KN], I32, kind="ExternalInput").ap()
    w_q = nc.dram_tensor("Wq", [D, D], F32, kind="ExternalInput").ap()
    b_q = nc.dram_tensor("bq", [1, D], F32, kind="ExternalInput").ap()
    w_k = nc.dram_tensor("Wk", [D, D], F32, kind="ExternalInput").ap()
    b_k = nc.dram_tensor("bk", [1, D], F32, kind="ExternalInput").ap()
    w_v = nc.dram_tensor("Wv", [D, D], F32, kind="ExternalInput").ap()
    b_v = nc.dram_tensor("bv", [1, D], F32, kind="ExternalInput").ap()

    ctx_d = nc.dram_tensor("ctx", [QN, D], F32, kind="ExternalOutput").ap()
    attn_d = nc.dram_tensor("attn", [QN, KN], F32, kind="ExternalOutput").ap()


    cb_toggle = [0]  # alternate ACT/DVE for PSUM->SBUF copybacks

    def copyback(dst, src):
        if cb_toggle[0] % 2 == 0:
            nc.scalar.activation(dst, src, AF.Copy)
        else:
            nc.vector.tensor_copy(dst, src)
        cb_toggle[0] += 1

    with tile.TileContext(nc, pool_alloc_mode="queue") as tc, ExitStack() as top:
        const_pool = top.enter_context(tc.tile_pool(name="consts", bufs=1))
        # ident_bf tile is created here but its instructions are emitted
        # after the first load DMA trigger so the GpSimd ring starts on DMAs
        ident_bf = const_pool.tile([P, P], BF16)
        zero_bias = const_pool.tile([P, 1], F32)
        nc.vector.memset(zero_bias[:], 0.0)
        bias_q = const_pool.tile([P, DC], F32)
        bias_k = const_pool.tile([P, DC], F32)

        ph1 = top.enter_context(ExitStack())
        xnat_pool = ph1.enter_context(tc.tile_pool(name="xnat", bufs=4))
        wnat_pool = ph1.enter_context(tc.tile_pool(name="wnat", bufs=1))

        def transpose_panel(x_dram, xT, s4, ps_tr):
            """Transpose 512 rows of x (f32, cast to bf16 by the DMA) into
            xT[:, :, s4*512:(s4+1)*512]."""
            xnat = xnat_pool.tile([P, 4, D], BF16, tag="xnat")
            nc.gpsimd.dma_start(
                out=xnat[:],
                in_=x_dram[s4 * 512:(s4 + 1) * 512, :]
                    .rearrange("(s p) c -> p s c", p=P))
            for cc2 in range(DC // 2):
                pt = ps_tr.tile([P, 8, P], BF16, tag="pt")
                for m in range(2):
                    cc = 2 * cc2 + m
                    for j in range(4):
                        nc.tensor.transpose(
                            pt[:, 4 * m + j, :],
                            xnat[:, j, cc * P:(cc + 1) * P], ident_bf[:])
                copyback(xT[:, 2 * cc2:2 * cc2 + 2, s4 * 512:(s4 + 1) * 512],
                         pt[:, :, :].rearrange("p (m j) f -> p m (j f)", m=2))

        def weights_T(w_dram, wt, ps_tr):
            """W [d, c] f32 natural -(cast DMA)-> bf16 -> wt [c_in_cc, cc, d]."""
            w_nat = wnat_pool.tile([P, DC, D], BF16, tag="wnat")
            nc.gpsimd.dma_start(
                out=w_nat[:],
                in_=w_dram.rearrange("(dc p) c -> p dc c", p=P))
            for cc in range(DC):
                pt = ps_tr.tile([P, 8, P], BF16, tag="pt")
                for dc in range(DC):
                    nc.tensor.transpose(
                        pt[:, dc, :],
                        w_nat[:, dc, cc * P:(cc + 1) * P], ident_bf[:])
                copyback(wt[:, cc, :], pt[:, :, :])

        # pqT, pv and pkt live through phase 2 (right-side pool stack)
        ph2 = top.enter_context(ExitStack())
        pqt_pool = ph2.enter_context(tc.tile_pool(name="pqt", bufs=1, side="right"))
        pv_pool = ph2.enter_context(tc.tile_pool(name="pv", bufs=1, side="right"))
        pkt_pool = ph2.enter_context(tc.tile_pool(name="pkt", bufs=1, side="right"))

        # ---- q step: pq^T resident (bf16) ----
        with ExitStack() as ph:
            ps_tr = ph.enter_context(tc.tile_pool(name="ps_tr_q", bufs=2, space="PSUM"))
            ps_pr = ph.enter_context(tc.tile_pool(name="ps_pr_q", bufs=3, space="PSUM"))
            wtp = ph.enter_context(tc.tile_pool(name="wt_q", bufs=1))
            wt = wtp.tile([P, DC, D], BF16)
            # first load DMA on the ring, then the identity build overlaps it
            w_nat_q = wnat_pool.tile([P, DC, D], BF16, tag="wnat")
            nc.gpsimd.dma_start(
                out=w_nat_q[:], in_=w_q.rearrange("(dc p) c -> p dc c", p=P))
            masks.make_identity(nc, ident_bf[:])
            for cc in range(DC):
                pt = ps_tr.tile([P, 8, P], BF16, tag="pt")
                for dc in range(DC):
                    nc.tensor.transpose(
                        pt[:, dc, :],
                        w_nat_q[:, dc, cc * P:(cc + 1) * P], ident_bf[:])
                copyback(wt[:, cc, :], pt[:, :, :])
            nc.gpsimd.dma_start(out=bias_q[:], in_=bass.AP(
                tensor=b_q.tensor, offset=b_q.offset, ap=[[1, P], [P, DC]]))
            nc.gpsimd.dma_start(out=bias_k[:], in_=bass.AP(
                tensor=b_k.tensor, offset=b_k.offset, ap=[[1, P], [P, DC]]))
            xtp = ph.enter_context(tc.tile_pool(name="xt_q", bufs=1))
            xT = xtp.tile([P, DC, QN], BF16)
            pqt = pqt_pool.tile([P, DC, QN], BF16)
            transpose_panel(q_d, xT, 0, ps_tr)
            for half in range(2):
                transpose_panel(q_d, xT, 2 * half + 1, ps_tr)
                if half == 0:
                    transpose_panel(q_d, xT, 2, ps_tr)
                for dc in range(DC):
                    ps_a = ps_pr.tile([P, 2, 512], F32, tag="ps_proj")
                    for cc in range(DC):
                        for n2 in range(2):
                            nc.tensor.matmul(
                                ps_a[:, n2, :],
                                lhsT=wt[:, cc, dc * P:(dc + 1) * P],
                                rhs=xT[:, cc, (2 * half + n2) * 512:(2 * half + n2 + 1) * 512],
                                start=(cc == 0), stop=(cc == DC - 1))
                    for n2 in range(2):
                        nc.scalar.activation(
                            pqt[:, dc, (2 * half + n2) * 512:(2 * half + n2 + 1) * 512],
                            ps_a[:, n2, :], AF.Identity,
                            bias=bias_q[:, dc:dc + 1], scale=1.0)

        # ---- v step: pv [k_in_s, s, d] bf16, resident ----
        with ExitStack() as ph:
            bvp = ph.enter_context(tc.tile_pool(name="bv", bufs=1))
            bv_bc = bvp.tile([P, D], F32)
            nc.gpsimd.dma_start(out=bv_bc[:], in_=b_v.to_broadcast([P, D]))
            ps_tr = ph.enter_context(tc.tile_pool(name="ps_tr_v", bufs=2, space="PSUM"))
            ps_pr = ph.enter_context(tc.tile_pool(name="ps_pr_v", bufs=3, space="PSUM"))
            wtp = ph.enter_context(tc.tile_pool(name="wt_v", bufs=1))
            wvt = wtp.tile([P, DC, D], BF16)
            weights_T(w_v, wvt, ps_tr)
            xtp = ph.enter_context(tc.tile_pool(name="vt", bufs=1))
            vT = xtp.tile([P, DC, KN], BF16)

            pv = pv_pool.tile([P, NS, D], BF16)
            for s4 in range(4):
                transpose_panel(v_d, vT, s4, ps_tr)
                for s in range(4 * s4, 4 * s4 + 4):
                    ps_v = ps_pr.tile([P, 2, 512], F32, tag="ps_proj")
                    for cc in range(DC):
                        for d2 in range(2):
                            nc.tensor.matmul(
                                ps_v[:, d2, :],
                                lhsT=vT[:, cc, s * P:(s + 1) * P],
                                rhs=wvt[:, cc, d2 * 512:(d2 + 1) * 512],
                                start=(cc == 0), stop=(cc == DC - 1))
                    for d2 in range(2):
                        nc.vector.scalar_tensor_tensor(
                            out=pv[:, s, d2 * 512:(d2 + 1) * 512],
                            in0=ps_v[:, d2, :], scalar=1.0,
                            in1=bv_bc[:, d2 * 512:(d2 + 1) * 512],
                            op0=ALU.mult, op1=ALU.add)

        # ---- k step: pk^T [d_in_dc, dc, n] bf16, resident ----
        with ExitStack() as ph:
            ps_tr = ph.enter_context(tc.tile_pool(name="ps_tr_k", bufs=2, space="PSUM"))
            ps_pr = ph.enter_context(tc.tile_pool(name="ps_pr_k", bufs=3, space="PSUM"))
            wtp = ph.enter_context(tc.tile_pool(name="wt_k", bufs=1))
            wkt = wtp.tile([P, DC, D], BF16)
            weights_T(w_k, wkt, ps_tr)
            xtp = ph.enter_context(tc.tile_pool(name="kt", bufs=1))
            kT = xtp.tile([P, DC, KN], BF16)

            pkt = pkt_pool.tile([P, DC, KN], BF16)
            transpose_panel(k_d, kT, 0, ps_tr)
            for half in range(2):
                transpose_panel(k_d, kT, 2 * half + 1, ps_tr)
                if half == 0:
                    transpose_panel(k_d, kT, 2, ps_tr)
                for dc in range(DC):
                    ps_a = ps_pr.tile([P, 2, 512], F32, tag="ps_proj")
                    for cc in range(DC):
                        for n2 in range(2):
                            nc.tensor.matmul(
                                ps_a[:, n2, :],
                                lhsT=wkt[:, cc, dc * P:(dc + 1) * P],
                                rhs=kT[:, cc, (2 * half + n2) * 512:(2 * half + n2 + 1) * 512],
                                start=(cc == 0), stop=(cc == DC - 1))
                    for n2 in range(2):
                        nc.scalar.activation(
                            pkt[:, dc, (2 * half + n2) * 512:(2 * half + n2 + 1) * 512],
                            ps_a[:, n2, :], AF.Identity,
                            bias=bias_k[:, dc:dc + 1], scale=1.0)

        ph1.close()

        # ---------------- Phase 2: attention, per query panel ----------------
        with ExitStack() as ph:
            maskp = ph.enter_context(tc.tile_pool(name="maskp", bufs=1))
            mask_f = maskp.tile([P, KN], BF16)
            with tc.tile_pool(name="mtmp", bufs=1) as mtmp:
                mask_i = mtmp.tile([P, KN], I32)
                nc.gpsimd.dma_start(out=mask_i[:], in_=m_d.to_broadcast([P, KN]))
                nc.vector.tensor_copy(mask_f[:], mask_i[:])

            e_pool = ph.enter_context(tc.tile_pool(name="em", bufs=2))
            attn_pool = ph.enter_context(tc.tile_pool(name="attnsb", bufs=2))
            at_pool = ph.enter_context(tc.tile_pool(name="attnT", bufs=2))
            ctx_pool = ph.enter_context(tc.tile_pool(name="ctxsb", bufs=2))
            small = ph.enter_context(tc.tile_pool(name="small", bufs=4))
            ps_s = ph.enter_context(tc.tile_pool(name="ps_s", bufs=2, space="PSUM"))
            ps_t = ph.enter_context(tc.tile_pool(name="ps_t", bufs=2, space="PSUM"))
            ps_c = ph.enter_context(tc.tile_pool(name="ps_c", bufs=1, space="PSUM"))

            def scores_half(p, h):
                ps = ps_s.tile([P, 2, 512], F32)
                for dc in range(DC):
                    for n2 in range(2):
                        nc.tensor.matmul(
                            ps[:, n2, :],
                            lhsT=pqt[:, dc, p * P:(p + 1) * P],
                            rhs=pkt[:, dc, (2 * h + n2) * 512:(2 * h + n2 + 1) * 512],
                            start=(dc == 0), stop=(dc == DC - 1))
                return ps

            s_half = [scores_half(0, 0), scores_half(0, 1)]

            for p in range(NPANEL):
                # softmax for panel p (ACT/DVE; overlaps next scores on PE)
                em = e_pool.tile([P, KN], BF16)
                for h in range(2):
                    nc.scalar.activation(
                        em[:, h * 1024:(h + 1) * 1024],
                        s_half[h][:, :, :], AF.Exp,
                        bias=zero_bias[:], scale=SCALE)
                den = small.tile([P, 1], F32)
                nc.vector.scalar_tensor_tensor(
                    out=em[:], in0=em[:], scalar=1.0, in1=mask_f[:],
                    op0=ALU.mult, op1=ALU.mult, accum_out=den[:])
                rden = small.tile([P, 1], F32)
                nc.vector.reciprocal(rden[:], den[:])

                if p + 1 < NPANEL:
                    ns0 = scores_half(p + 1, 0)

                # attn output row-panel (f32)
                attn_sb = attn_pool.tile([P, KN], F32)
                nc.vector.tensor_scalar_mul(attn_sb[:], em[:], rden[:])
                nc.scalar.dma_start(
                    out=attn_d[p * P:(p + 1) * P, :], in_=attn_sb[:])

                # transpose Em (bf16) -> attnT [k_in_s, s, nq]; the 1/den
                # normalization is folded into the ctx copyback scale
                attnT = at_pool.tile([P, NS, P], BF16)
                for g in range(2):
                    pt = ps_t.tile([P, 8, P], BF16)
                    for j in range(8):
                        s_idx = g * 8 + j
                        nc.tensor.transpose(
                            pt[:, j, :],
                            em[:, s_idx * P:(s_idx + 1) * P], ident_bf[:])
                    copyback(attnT[:, g * 8:(g + 1) * 8, :], pt[:, :, :])

                # ctx panel
                ps_ctx = ps_c.tile([P, 2, 512], F32)
                for s in range(NS):
                    for d2 in range(2):
                        nc.tensor.matmul(
                            ps_ctx[:, d2, :],
                            lhsT=attnT[:, s, :],
                            rhs=pv[:, s, d2 * 512:(d2 + 1) * 512],
                            start=(s == 0), stop=(s == NS - 1))
                ctx_sb = ctx_pool.tile([P, D], F32)
                nc.scalar.activation(ctx_sb[:], ps_ctx[:, :, :], AF.Copy,
                                     scale=rden[:])
                nc.scalar.dma_start(
                    out=ctx_d[p * P:(p + 1) * P, :], in_=ctx_sb[:])

                if p + 1 < NPANEL:
                    ns1 = scores_half(p + 1, 1)
                    s_half = [ns0, ns1]

    nc.compile()
    return nc


def _get_nc():
    global _cached_nc
    if _cached_nc is None:
        _cached_nc = _build()
    return _cached_nc


last_exec_time_ns = None


def kernel(**inputs):
    global last_exec_time_ns
    nc = _get_nc()
    query = np.asarray(inputs["query"], dtype=np.float32)
    key = np.asarray(inputs["key"], dtype=np.float32)
    value = np.asarray(inputs["value"], dtype=np.float32)
    mask = np.asarray(inputs["mask"], dtype=np.int32)
    Wq = np.ascontiguousarray(np.asarray(inputs["Wq"], dtype=np.float32))
    bq = np.asarray(inputs["bq"], dtype=np.float32).reshape(1, D)
    Wk = np.ascontiguousarray(np.asarray(inputs["Wk"], dtype=np.float32))
    bk = np.asarray(inputs["bk"], dtype=np.float32).reshape(1, D)
    Wv = np.ascontiguousarray(np.asarray(inputs["Wv"], dtype=np.float32))
    bv = np.asarray(inputs["bv"], dtype=np.float32).reshape(1, D)

    in_maps = []
    for b in range(B):
        in_maps.append({
            "query": np.ascontiguousarray(query[b]),
            "key": np.ascontiguousarray(key[b]),
            "value": np.ascontiguousarray(value[b]),
            "mask": np.ascontiguousarray(mask[b].reshape(1, KN)),
            "Wq": Wq, "bq": bq, "Wk": Wk, "bk": bk, "Wv": Wv, "bv": bv,
        })

    trace = bool(os.environ.get("ATTN_TRACE"))
    res = None
    last_err = None
    for attempt in range(3):
        try:
            res = run_bass_kernel_spmd(nc, in_maps,
                                       core_ids=list(range(N_CORES)),
                                       trace=trace and attempt == 0)
            break
        except Exception as e:  # transient NRT device errors: retry
            last_err = e
    if res is None:
        raise last_err
    last_exec_time_ns = res.exec_time_ns

    ctx = np.stack([res.results[b]["ctx"] for b in range(B)])
    attn = np.stack([res.results[b]["attn"] for b in range(B)])
    return (ctx, attn)


# revision 32
# speedup vs baseline: 1.1893x; 1.0058x over previous
"""Fused attention layer (projections + masked softmax + context) on 8 TRN2
NeuronCores, data-parallel over batch (one batch element per core).

Per core (batch b):
  pq = q @ Wq.T + bq ; pk = k @ Wk.T + bk ; pv = v @ Wv.T + bv
  scores = pq @ pk.T / 32 ;  E = exp(scores) * mask
  attn = E / rowsum(E) ;  ctx = attn @ pv

Matmuls contract over the SBUF partition dim, so activations/weights are
transposed on-chip via TensorE (identity matmul) in bf16; the f32 -> bf16
cast rides the SWDGE load DMAs for free. All matmul operands are bf16
(full PE rate; FWL weight loads); PSUM accumulation and the softmax are
f32. pq^T, pk^T and pv are all bf16 and stay fully resident in SBUF
across phase 2 (the projection epilogues write them in place; no DRAM
round trips). Phase 2 is software-pipelined: scores of panel p+1 run on
TensorE while softmax of panel p runs on ScalarE/VectorE, and the 1/den
normalization of ctx is folded into the PSUM->SBUF copyback scale.
"""
import os
from contextlib import ExitStack

import numpy as np

import concourse.bass as bass
import concourse.tile as tile
from concourse import bacc, mybir, masks
from concourse.bass_utils import run_bass_kernel_spmd

F32 = mybir.dt.float32
BF16 = mybir.dt.bfloat16
I32 = mybir.dt.int32
AF = mybir.ActivationFunctionType
ALU = mybir.AluOpType

P = 128
B = 8
QN = 2048
KN = 2048
D = 1024           # DIN == DPROJ
DC = D // P        # 8 contraction chunks
NPANEL = QN // P   # 16 query panels
NS = KN // P       # 16 key chunks
SCALE = 1.0 / 32.0 # 1/sqrt(DPROJ)

N_CORES = 8

_cached_nc = None


def _build():
    nc = bacc.Bacc("TRN2", target_bir_lowering=False, debug=False,
                   num_devices=N_CORES)

    q_d = nc.dram_tensor("query", [QN, D], F32, kind="ExternalInput").ap()
    k_d = nc.dram_tensor("key", [KN, D], F32, kind="ExternalInput").ap()
    v_d = nc.dram_tensor("value", [KN, D], F32, kind="ExternalInput").ap()
    m_d = nc.dram_tensor("mask", [1, KN], I32, kind="ExternalInput").ap()
    w_q = nc.dram_tensor("Wq", [D, D], F32, kind="ExternalInput").ap()
    b_q = nc.dram_tensor("bq", [1, D], F32, kind="ExternalInput").ap()
    w_k = nc.dram_tensor("Wk", [D, D], F32, kind="ExternalInput").ap()
    b_k = nc.dram_tensor("bk", [1, D], F32, kind="ExternalInput").ap()
    w_v = nc.dram_tensor("Wv", [D, D], F32, kind="ExternalInput").ap()
    b_v = nc.dram_tensor("bv", [1, D], F32, kind="ExternalInput").ap()

    ctx_d = nc.dram_tensor("ctx", [QN, D], F32, kind="ExternalOutput").ap()
    attn_d = nc.dram_tensor("attn", [QN, KN], F32, kind="ExternalOutput").ap()


    cb_toggle = [0]  # alternate ACT/DVE for PSUM->SBUF copybacks

    def copyback(dst, src):
        if cb_toggle[0] % 2 == 0:
            nc.scalar.activation(dst, src, AF.Copy)
        else:
            nc.vector.tensor_copy(dst, src)
        cb_toggle[0] += 1

    with tile.TileContext(nc, pool_alloc_mode="queue") as tc, ExitStack() as top:
        const_pool = top.enter_context(tc.tile_pool(name="consts", bufs=1))
        # ident_bf tile is created here but its instructions are emitted
        # after the first load DMA trigger so the GpSimd ring starts on DMAs
        ident_bf = const_pool.tile([P, P], BF16)
        zero_bias = const_pool.tile([P, 1], F32)
        nc.vector.memset(zero_bias[:], 0.0)
        bias_q = const_pool.tile([P, DC], F32)
        bias_k = const_pool.tile([P, DC], F32)

        ph1 = top.enter_context(ExitStack())
        xnat_pool = ph1.enter_context(tc.tile_pool(name="xnat", bufs=3))
        wnat_pool = ph1.enter_context(tc.tile_pool(name="wnat", bufs=2))

        def transpose_panel(x_dram, xT, s4, ps_tr):
            """Transpose 512 rows of x (f32, cast to bf16 by the DMA) into
            xT[:, :, s4*512:(s4+1)*512]."""
            xnat = xnat_pool.tile([P, 4, D], BF16, tag="xnat")
            nc.gpsimd.dma_start(
                out=xnat[:],
                in_=x_dram[s4 * 512:(s4 + 1) * 512, :]
                    .rearrange("(s p) c -> p s c", p=P))
            for cc2 in range(DC // 2):
                pt = ps_tr.tile([P, 8, P], BF16, tag="pt")
                for m in range(2):
                    cc = 2 * cc2 + m
                    for j in range(4):
                        nc.tensor.transpose(
                            pt[:, 4 * m + j, :],
                            xnat[:, j, cc * P:(cc + 1) * P], ident_bf[:])
                copyback(xT[:, 2 * cc2:2 * cc2 + 2, s4 * 512:(s4 + 1) * 512],
                         pt[:, :, :].rearrange("p (m j) f -> p m (j f)", m=2))

        def weights_T(w_dram, wt, ps_tr):
            """W [d, c] f32 natural -(cast DMA)-> bf16 -> wt [c_in_cc, cc, d]."""
            w_nat = wnat_pool.tile([P, DC, D], BF16, tag="wnat")
            nc.gpsimd.dma_start(
                out=w_nat[:],
                in_=w_dram.rearrange("(dc p) c -> p dc c", p=P))
            for cc in range(DC):
                pt = ps_tr.tile([P, 8, P], BF16, tag="pt")
                for dc in range(DC):
                    nc.tensor.transpose(
                        pt[:, dc, :],
                        w_nat[:, dc, cc * P:(cc + 1) * P], ident_bf[:])
                copyback(wt[:, cc, :], pt[:, :, :])

        # pqT, pv and pkt live through phase 2 (right-side pool stack)
        ph2 = top.enter_context(ExitStack())
        pqt_pool = ph2.enter_context(tc.tile_pool(name="pqt", bufs=1, side="right"))
        pv_pool = ph2.enter_context(tc.tile_pool(name="pv", bufs=1, side="right"))
        pkt_pool = ph2.enter_context(tc.tile_pool(name="pkt", bufs=1, side="right"))

        # ---- q step: pq^T resident (bf16) ----
        with ExitStack() as ph:
            ps_tr = ph.enter_context(tc.tile_pool(name="ps_tr_q", bufs=2, space="PSUM"))
            ps_pr = ph.enter_context(tc.tile_pool(name="ps_pr_q", bufs=3, space="PSUM"))
            wtp = ph.enter_context(tc.tile_pool(name="wt_q", bufs=1))
            wt = wtp.tile([P, DC, D], BF16)
            # first load DMA on the ring, then the identity build overlaps it
            w_nat_q = wnat_pool.tile([P, DC, D], BF16, tag="wnat")
            nc.gpsimd.dma_start(
                out=w_nat_q[:], in_=w_q.rearrange("(dc p) c -> p dc c", p=P))
            masks.make_identity(nc, ident_bf[:])
            for cc in range(DC):
                pt = ps_tr.tile([P, 8, P], BF16, tag="pt")
                for dc in range(DC):
                    nc.tensor.transpose(
                        pt[:, dc, :],
                        w_nat_q[:, dc, cc * P:(cc + 1) * P], ident_bf[:])
                copyback(wt[:, cc, :], pt[:, :, :])
            nc.gpsimd.dma_start(out=bias_q[:], in_=bass.AP(
                tensor=b_q.tensor, offset=b_q.offset, ap=[[1, P], [P, DC]]))
            nc.gpsimd.dma_start(out=bias_k[:], in_=bass.AP(
                tensor=b_k.tensor, offset=b_k.offset, ap=[[1, P], [P, DC]]))
            xtp = ph.enter_context(tc.tile_pool(name="xt_q", bufs=1))
            xT = xtp.tile([P, DC, QN], BF16)
            pqt = pqt_pool.tile([P, DC, QN], BF16)
            transpose_panel(q_d, xT, 0, ps_tr)
            for half in range(2):
                transpose_panel(q_d, xT, 2 * half + 1, ps_tr)
                if half == 0:
                    transpose_panel(q_d, xT, 2, ps_tr)
                for dc in range(DC):
                    ps_a = ps_pr.tile([P, 2, 512], F32, tag="ps_proj")
                    for cc in range(DC):
                        for n2 in range(2):
                            nc.tensor.matmul(
                                ps_a[:, n2, :],
                                lhsT=wt[:, cc, dc * P:(dc + 1) * P],
                                rhs=xT[:, cc, (2 * half + n2) * 512:(2 * half + n2 + 1) * 512],
                                start=(cc == 0), stop=(cc == DC - 1))
                    for n2 in range(2):
                        nc.scalar.activation(
                            pqt[:, dc, (2 * half + n2) * 512:(2 * half + n2 + 1) * 512],
                            ps_a[:, n2, :], AF.Identity,
                            bias=bias_q[:, dc:dc + 1], scale=1.0)

        # ---- v step: pv [k_in_s, s, d] bf16, resident ----
        with ExitStack() as ph:
            bvp = ph.enter_context(tc.tile_pool(name="bv", bufs=1))
            bv_bc = bvp.tile([P, D], F32)
            nc.gpsimd.dma_start(out=bv_bc[:], in_=b_v.to_broadcast([P, D]))
            ps_tr = ph.enter_context(tc.tile_pool(name="ps_tr_v", bufs=2, space="PSUM"))
            ps_pr = ph.enter_context(tc.tile_pool(name="ps_pr_v", bufs=3, space="PSUM"))
            wtp = ph.enter_context(tc.tile_pool(name="wt_v", bufs=1))
            wvt = wtp.tile([P, DC, D], BF16)
            weights_T(w_v, wvt, ps_tr)
            xtp = ph.enter_context(tc.tile_pool(name="vt", bufs=1))
            vT = xtp.tile([P, DC, KN], BF16)

            pv = pv_pool.tile([P, NS, D], BF16)
            for s4 in range(4):
                transpose_panel(v_d, vT, s4, ps_tr)
                for s in range(4 * s4, 4 * s4 + 4):
                    ps_v = ps_pr.tile([P, 2, 512], F32, tag="ps_proj")
                    for cc in range(DC):
                        for d2 in range(2):
                            nc.tensor.matmul(
                                ps_v[:, d2, :],
                                lhsT=vT[:, cc, s * P:(s + 1) * P],
                                rhs=wvt[:, cc, d2 * 512:(d2 + 1) * 512],
                                start=(cc == 0), stop=(cc == DC - 1))
                    for d2 in range(2):
                        nc.vector.scalar_tensor_tensor(
                            out=pv[:, s, d2 * 512:(d2 + 1) * 512],
                            in0=ps_v[:, d2, :], scalar=1.0,
                            in1=bv_bc[:, d2 * 512:(d2 + 1) * 512],
                            op0=ALU.mult, op1=ALU.add)

        # ---- k step: pk^T [d_in_dc, dc, n] bf16, resident ----
        with ExitStack() as ph:
            ps_tr = ph.enter_context(tc.tile_pool(name="ps_tr_k", bufs=2, space="PSUM"))
            ps_pr = ph.enter_context(tc.tile_pool(name="ps_pr_k", bufs=3, space="PSUM"))
            wtp = ph.enter_context(tc.tile_pool(name="wt_k", bufs=1))
            wkt = wtp.tile([P, DC, D], BF16)
            weights_T(w_k, wkt, ps_tr)
            xtp = ph.enter_context(tc.tile_pool(name="kt", bufs=1))
            kT = xtp.tile([P, DC, KN], BF16)

            pkt = pkt_pool.tile([P, DC, KN], BF16)
            transpose_panel(k_d, kT, 0, ps_tr)
            for half in range(2):
                transpose_panel(k_d, kT, 2 * half + 1, ps_tr)
                if half == 0:
                    transpose_panel(k_d, kT, 2, ps_tr)
                for dc in range(DC):
                    ps_a = ps_pr.tile([P, 2, 512], F32, tag="ps_proj")
                    for cc in range(DC):
                        for n2 in range(2):
                            nc.tensor.matmul(
                                ps_a[:, n2, :],
                                lhsT=wkt[:, cc, dc * P:(dc + 1) * P],
                                rhs=kT[:, cc, (2 * half + n2) * 512:(2 * half + n2 + 1) * 512],
                                start=(cc == 0), stop=(cc == DC - 1))
                    for n2 in range(2):
                        nc.scalar.activation(
                            pkt[:, dc, (2 * half + n2) * 512:(2 * half + n2 + 1) * 512],
                            ps_a[:, n2, :], AF.Identity,
                            bias=bias_k[:, dc:dc + 1], scale=1.0)

        ph1.close()

        # ---------------- Phase 2: attention, per query panel ----------------
        with ExitStack() as ph:
            maskp = ph.enter_context(tc.tile_pool(name="maskp", bufs=1))
            mask_f = maskp.tile([P, KN], BF16)
            with tc.tile_pool(name="mtmp", bufs=1) as mtmp:
                mask_i = mtmp.tile([P, KN], I32)
                nc.gpsimd.dma_start(out=mask_i[:], in_=m_d.to_broadcast([P, KN]))
                nc.vector.tensor_copy(mask_f[:], mask_i[:])

            e_pool = ph.enter_context(tc.tile_pool(name="em", bufs=2))
            attn_pool = ph.enter_context(tc.tile_pool(name="attnsb", bufs=2))
            at_pool = ph.enter_context(tc.tile_pool(name="attnT", bufs=2))
            ctx_pool = ph.enter_context(tc.tile_pool(name="ctxsb", bufs=2))
            small = ph.enter_context(tc.tile_pool(name="small", bufs=4))
            ps_s = ph.enter_context(tc.tile_pool(name="ps_s", bufs=2, space="PSUM"))
            ps_t = ph.enter_context(tc.tile_pool(name="ps_t", bufs=2, space="PSUM"))
            ps_c = ph.enter_context(tc.tile_pool(name="ps_c", bufs=1, space="PSUM"))

            def scores_half(p, h):
                ps = ps_s.tile([P, 2, 512], F32)
                for dc in range(DC):
                    for n2 in range(2):
                        nc.tensor.matmul(
                            ps[:, n2, :],
                            lhsT=pqt[:, dc, p * P:(p + 1) * P],
                            rhs=pkt[:, dc, (2 * h + n2) * 512:(2 * h + n2 + 1) * 512],
                            start=(dc == 0), stop=(dc == DC - 1))
                return ps

            s_half = [scores_half(0, 0), scores_half(0, 1)]

            for p in range(NPANEL):
                # softmax for panel p (ACT/DVE; overlaps next scores on PE)
                em = e_pool.tile([P, KN], BF16)
                for h in range(2):
                    nc.scalar.activation(
                        em[:, h * 1024:(h + 1) * 1024],
                        s_half[h][:, :, :], AF.Exp,
                        bias=zero_bias[:], scale=SCALE)
                den = small.tile([P, 1], F32)
                nc.vector.scalar_tensor_tensor(
                    out=em[:], in0=em[:], scalar=1.0, in1=mask_f[:],
                    op0=ALU.mult, op1=ALU.mult, accum_out=den[:])
                rden = small.tile([P, 1], F32)
                nc.vector.reciprocal(rden[:], den[:])

                if p + 1 < NPANEL:
                    ns0 = scores_half(p + 1, 0)

                # attn output row-panel (f32)
                attn_sb = attn_pool.tile([P, KN], F32)
                nc.vector.tensor_scalar_mul(attn_sb[:], em[:], rden[:])
                nc.scalar.dma_start(
                    out=attn_d[p * P:(p + 1) * P, :], in_=attn_sb[:])

                # transpose Em (bf16) -> attnT [k_in_s, s, nq]; the 1/den
                # normalization is folded into the ctx copyback scale
                attnT = at_pool.tile([P, NS, P], BF16)
                for g in range(2):
                    pt = ps_t.tile([P, 8, P], BF16)
                    for j in range(8):
                        s_idx = g * 8 + j
                        nc.tensor.transpose(
                            pt[:, j, :],
                            em[:, s_idx * P:(s_idx + 1) * P], ident_bf[:])
                    copyback(attnT[:, g * 8:(g + 1) * 8, :], pt[:, :, :])

                # ctx panel
                ps_ctx = ps_c.tile([P, 2, 512], F32)
                for s in range(NS):
                    for d2 in range(2):
                        nc.tensor.matmul(
                            ps_ctx[:, d2, :],
                            lhsT=attnT[:, s, :],
                            rhs=pv[:, s, d2 * 512:(d2 + 1) * 512],
                            start=(s == 0), stop=(s == NS - 1))
                ctx_sb = ctx_pool.tile([P, D], F32)
                nc.scalar.activation(ctx_sb[:], ps_ctx[:, :, :], AF.Copy,
                                     scale=rden[:])
                nc.scalar.dma_start(
                    out=ctx_d[p * P:(p + 1) * P, :], in_=ctx_sb[:])

                if p + 1 < NPANEL:
                    ns1 = scores_half(p + 1, 1)
                    s_half = [ns0, ns1]

    nc.compile()
    return nc


def _get_nc():
    global _cached_nc
    if _cached_nc is None:
        _cached_nc = _build()
    return _cached_nc


last_exec_time_ns = None


def kernel(**inputs):
    global last_exec_time_ns
    nc = _get_nc()
    query = np.asarray(inputs["query"], dtype=np.float32)
    key = np.asarray(inputs["key"], dtype=np.float32)
    value = np.asarray(inputs["value"], dtype=np.float32)
    mask = np.asarray(inputs["mask"], dtype=np.int32)
    Wq = np.ascontiguousarray(np.asarray(inputs["Wq"], dtype=np.float32))
    bq = np.asarray(inputs["bq"], dtype=np.float32).reshape(1, D)
    Wk = np.ascontiguousarray(np.asarray(inputs["Wk"], dtype=np.float32))
    bk = np.asarray(inputs["bk"], dtype=np.float32).reshape(1, D)
    Wv = np.ascontiguousarray(np.asarray(inputs["Wv"], dtype=np.float32))
    bv = np.asarray(inputs["bv"], dtype=np.float32).reshape(1, D)

    in_maps = []
    for b in range(B):
        in_maps.append({
            "query": np.ascontiguousarray(query[b]),
            "key": np.ascontiguousarray(key[b]),
            "value": np.ascontiguousarray(value[b]),
            "mask": np.ascontiguousarray(mask[b].reshape(1, KN)),
            "Wq": Wq, "bq": bq, "Wk": Wk, "bk": bk, "Wv": Wv, "bv": bv,
        })

    trace = bool(os.environ.get("ATTN_TRACE"))
    res = None
    last_err = None
    for attempt in range(3):
        try:
            res = run_bass_kernel_spmd(nc, in_maps,
                                       core_ids=list(range(N_CORES)),
                                       trace=trace and attempt == 0)
            break
        except Exception as e:  # transient NRT device errors: retry
            last_err = e
    if res is None:
        raise last_err
    last_exec_time_ns = res.exec_time_ns

    ctx = np.stack([res.results[b]["ctx"] for b in range(B)])
    attn = np.stack([res.results[b]["attn"] for b in range(B)])
    return (ctx, attn)


# revision 33
# speedup vs baseline: 1.2067x; 1.0147x over previous
"""Fused attention layer (projections + masked softmax + context) on 8 TRN2
NeuronCores, data-parallel over batch (one batch element per core).

Per core (batch b):
  pq = q @ Wq.T + bq ; pk = k @ Wk.T + bk ; pv = v @ Wv.T + bv
  scores = pq @ pk.T / 32 ;  E = exp(scores) * mask
  attn = E / rowsum(E) ;  ctx = attn @ pv

Matmuls contract over the SBUF partition dim, so activations/weights are
transposed on-chip via TensorE (identity matmul) in bf16; the f32 -> bf16
cast rides the SWDGE load DMAs for free. All matmul operands are bf16
(full PE rate; FWL weight loads); PSUM accumulation and the softmax are
f32. pq^T, pk^T and pv are all bf16 and stay fully resident in SBUF
across phase 2 (the projection epilogues write them in place; no DRAM
round trips). Phase 2 is software-pipelined: scores of panel p+1 run on
TensorE while softmax of panel p runs on ScalarE/VectorE, and the 1/den
normalization of ctx is folded into the PSUM->SBUF copyback scale.
"""
import os
from contextlib import ExitStack

import numpy as np

import concourse.bass as bass
import concourse.tile as tile
from concourse import bacc, mybir, masks
from concourse.bass_utils import run_bass_kernel_spmd

F32 = mybir.dt.float32
BF16 = mybir.dt.bfloat16
I32 = mybir.dt.int32
AF = mybir.ActivationFunctionType
ALU = mybir.AluOpType

P = 128
B = 8
QN = 2048
KN = 2048
D = 1024           # DIN == DPROJ
DC = D // P        # 8 contraction chunks
NPANEL = QN // P   # 16 query panels
NS = KN // P       # 16 key chunks
SCALE = 1.0 / 32.0 # 1/sqrt(DPROJ)

N_CORES = 8

_cached_nc = None


def _build():
    nc = bacc.Bacc("TRN2", target_bir_lowering=False, debug=False,
                   num_devices=N_CORES)

    q_d = nc.dram_tensor("query", [QN, D], F32, kind="ExternalInput").ap()
    k_d = nc.dram_tensor("key", [KN, D], F32, kind="ExternalInput").ap()
    v_d = nc.dram_tensor("value", [KN, D], F32, kind="ExternalInput").ap()
    m_d = nc.dram_tensor("mask", [1, KN], I32, kind="ExternalInput").ap()
    w_q = nc.dram_tensor("Wq", [D, D], F32, kind="ExternalInput").ap()
    b_q = nc.dram_tensor("bq", [1, D], F32, kind="ExternalInput").ap()
    w_k = nc.dram_tensor("Wk", [D, D], F32, kind="ExternalInput").ap()
    b_k = nc.dram_tensor("bk", [1, D], F32, kind="ExternalInput").ap()
    w_v = nc.dram_tensor("Wv", [D, D], F32, kind="ExternalInput").ap()
    b_v = nc.dram_tensor("bv", [1, D], F32, kind="ExternalInput").ap()

    ctx_d = nc.dram_tensor("ctx", [QN, D], F32, kind="ExternalOutput").ap()
    attn_d = nc.dram_tensor("attn", [QN, KN], F32, kind="ExternalOutput").ap()


    cb_toggle = [0]  # alternate ACT/DVE for PSUM->SBUF copybacks

    def copyback(dst, src):
        if cb_toggle[0] % 2 == 0:
            nc.scalar.activation(dst, src, AF.Copy)
        else:
            nc.vector.tensor_copy(dst, src)
        cb_toggle[0] += 1

    with tile.TileContext(nc, pool_alloc_mode="queue") as tc, ExitStack() as top:
        const_pool = top.enter_context(tc.tile_pool(name="consts", bufs=1))
        # ident_bf tile is created here but its instructions are emitted
        # after the first load DMA trigger so the GpSimd ring starts on DMAs
        ident_bf = const_pool.tile([P, P], BF16)
        zero_bias = const_pool.tile([P, 1], F32)
        nc.vector.memset(zero_bias[:], 0.0)
        bias_q = const_pool.tile([P, DC], F32)
        bias_k = const_pool.tile([P, DC], F32)

        ph1 = top.enter_context(ExitStack())
        xnat_pool = ph1.enter_context(tc.tile_pool(name="xnat", bufs=3))
        wnat_pool = ph1.enter_context(tc.tile_pool(name="wnat", bufs=2))

        def transpose_panel(x_dram, xT, s4, ps_tr):
            """Transpose 512 rows of x (f32, cast to bf16 by the DMA) into
            xT[:, :, s4*512:(s4+1)*512]."""
            xnat = xnat_pool.tile([P, 4, D], BF16, tag="xnat")
            nc.gpsimd.dma_start(
                out=xnat[:],
                in_=x_dram[s4 * 512:(s4 + 1) * 512, :]
                    .rearrange("(s p) c -> p s c", p=P))
            for cc2 in range(DC // 2):
                pt = ps_tr.tile([P, 8, P], BF16, tag="pt")
                for m in range(2):
                    cc = 2 * cc2 + m
                    for j in range(4):
                        nc.tensor.transpose(
                            pt[:, 4 * m + j, :],
                            xnat[:, j, cc * P:(cc + 1) * P], ident_bf[:])
                copyback(xT[:, 2 * cc2:2 * cc2 + 2, s4 * 512:(s4 + 1) * 512],
                         pt[:, :, :].rearrange("p (m j) f -> p m (j f)", m=2))

        def weights_T(w_dram, wt, ps_tr):
            """W [d, c] f32 natural -(cast DMA)-> bf16 -> wt [c_in_cc, cc, d]."""
            w_nat = wnat_pool.tile([P, DC, D], BF16, tag="wnat")
            nc.gpsimd.dma_start(
                out=w_nat[:],
                in_=w_dram.rearrange("(dc p) c -> p dc c", p=P))
            for cc in range(DC):
                pt = ps_tr.tile([P, 8, P], BF16, tag="pt")
                for dc in range(DC):
                    nc.tensor.transpose(
                        pt[:, dc, :],
                        w_nat[:, dc, cc * P:(cc + 1) * P], ident_bf[:])
                copyback(wt[:, cc, :], pt[:, :, :])

        # pqT, pv and pkt live through phase 2 (right-side pool stack)
        ph2 = top.enter_context(ExitStack())
        pqt_pool = ph2.enter_context(tc.tile_pool(name="pqt", bufs=1, side="right"))
        pv_pool = ph2.enter_context(tc.tile_pool(name="pv", bufs=1, side="right"))
        pkt_pool = ph2.enter_context(tc.tile_pool(name="pkt", bufs=1, side="right"))

        # ---- q step: pq^T resident (bf16) ----
        with ExitStack() as ph:
            ps_tr = ph.enter_context(tc.tile_pool(name="ps_tr_q", bufs=3, space="PSUM"))
            ps_pr = ph.enter_context(tc.tile_pool(name="ps_pr_q", bufs=2, space="PSUM"))
            wtp = ph.enter_context(tc.tile_pool(name="wt_q", bufs=1))
            wt = wtp.tile([P, DC, D], BF16)
            # first load DMA on the ring, then the identity build overlaps it
            w_nat_q = wnat_pool.tile([P, DC, D], BF16, tag="wnat")
            nc.gpsimd.dma_start(
                out=w_nat_q[:], in_=w_q.rearrange("(dc p) c -> p dc c", p=P))
            masks.make_identity(nc, ident_bf[:])
            for cc in range(DC):
                pt = ps_tr.tile([P, 8, P], BF16, tag="pt")
                for dc in range(DC):
                    nc.tensor.transpose(
                        pt[:, dc, :],
                        w_nat_q[:, dc, cc * P:(cc + 1) * P], ident_bf[:])
                copyback(wt[:, cc, :], pt[:, :, :])
            nc.gpsimd.dma_start(out=bias_q[:], in_=bass.AP(
                tensor=b_q.tensor, offset=b_q.offset, ap=[[1, P], [P, DC]]))
            nc.gpsimd.dma_start(out=bias_k[:], in_=bass.AP(
                tensor=b_k.tensor, offset=b_k.offset, ap=[[1, P], [P, DC]]))
            xtp = ph.enter_context(tc.tile_pool(name="xt_q", bufs=1))
            xT = xtp.tile([P, DC, QN], BF16)
            pqt = pqt_pool.tile([P, DC, QN], BF16)
            transpose_panel(q_d, xT, 0, ps_tr)
            for half in range(2):
                transpose_panel(q_d, xT, 2 * half + 1, ps_tr)
                if half == 0:
                    transpose_panel(q_d, xT, 2, ps_tr)
                for dc in range(DC):
                    ps_a = ps_pr.tile([P, 2, 512], F32, tag="ps_proj")
                    for cc in range(DC):
                        for n2 in range(2):
                            nc.tensor.matmul(
                                ps_a[:, n2, :],
                                lhsT=wt[:, cc, dc * P:(dc + 1) * P],
                                rhs=xT[:, cc, (2 * half + n2) * 512:(2 * half + n2 + 1) * 512],
                                start=(cc == 0), stop=(cc == DC - 1))
                    for n2 in range(2):
                        nc.scalar.activation(
                            pqt[:, dc, (2 * half + n2) * 512:(2 * half + n2 + 1) * 512],
                            ps_a[:, n2, :], AF.Identity,
                            bias=bias_q[:, dc:dc + 1], scale=1.0)

        # ---- v step: pv [k_in_s, s, d] bf16, resident ----
        with ExitStack() as ph:
            bvp = ph.enter_context(tc.tile_pool(name="bv", bufs=1))
            bv_bc = bvp.tile([P, D], F32)
            nc.gpsimd.dma_start(out=bv_bc[:], in_=b_v.to_broadcast([P, D]))
            ps_tr = ph.enter_context(tc.tile_pool(name="ps_tr_v", bufs=3, space="PSUM"))
            ps_pr = ph.enter_context(tc.tile_pool(name="ps_pr_v", bufs=2, space="PSUM"))
            wtp = ph.enter_context(tc.tile_pool(name="wt_v", bufs=1))
            wvt = wtp.tile([P, DC, D], BF16)
            weights_T(w_v, wvt, ps_tr)
            xtp = ph.enter_context(tc.tile_pool(name="vt", bufs=1))
            vT = xtp.tile([P, DC, KN], BF16)

            pv = pv_pool.tile([P, NS, D], BF16)
            for s4 in range(4):
                transpose_panel(v_d, vT, s4, ps_tr)
                for s in range(4 * s4, 4 * s4 + 4):
                    ps_v = ps_pr.tile([P, 2, 512], F32, tag="ps_proj")
                    for cc in range(DC):
                        for d2 in range(2):
                            nc.tensor.matmul(
                                ps_v[:, d2, :],
                                lhsT=vT[:, cc, s * P:(s + 1) * P],
                                rhs=wvt[:, cc, d2 * 512:(d2 + 1) * 512],
                                start=(cc == 0), stop=(cc == DC - 1))
                    for d2 in range(2):
                        nc.vector.scalar_tensor_tensor(
                            out=pv[:, s, d2 * 512:(d2 + 1) * 512],
                            in0=ps_v[:, d2, :], scalar=1.0,
                            in1=bv_bc[:, d2 * 512:(d2 + 1) * 512],
                            op0=ALU.mult, op1=ALU.add)

        # ---- k step: pk^T [d_in_dc, dc, n] bf16, resident ----
        with ExitStack() as ph:
            ps_tr = ph.enter_context(tc.tile_pool(name="ps_tr_k", bufs=3, space="PSUM"))
            ps_pr = ph.enter_context(tc.tile_pool(name="ps_pr_k", bufs=2, space="PSUM"))
            wtp = ph.enter_context(tc.tile_pool(name="wt_k", bufs=1))
            wkt = wtp.tile([P, DC, D], BF16)
            weights_T(w_k, wkt, ps_tr)
            xtp = ph.enter_context(tc.tile_pool(name="kt", bufs=1))
            kT = xtp.tile([P, DC, KN], BF16)

            pkt = pkt_pool.tile([P, DC, KN], BF16)
            transpose_panel(k_d, kT, 0, ps_tr)
            for half in range(2):
                transpose_panel(k_d, kT, 2 * half + 1, ps_tr)
                if half == 0:
                    transpose_panel(k_d, kT, 2, ps_tr)
                for dc in range(DC):
                    ps_a = ps_pr.tile([P, 2, 512], F32, tag="ps_proj")
                    for cc in range(DC):
                        for n2 in range(2):
                            nc.tensor.matmul(
                                ps_a[:, n2, :],
                                lhsT=wkt[:, cc, dc * P:(dc + 1) * P],
                                rhs=kT[:, cc, (2 * half + n2) * 512:(2 * half + n2 + 1) * 512],
                                start=(cc == 0), stop=(cc == DC - 1))
                    for n2 in range(2):
                        nc.scalar.activation(
                            pkt[:, dc, (2 * half + n2) * 512:(2 * half + n2 + 1) * 512],
                            ps_a[:, n2, :], AF.Identity,
                            bias=bias_k[:, dc:dc + 1], scale=1.0)

        ph1.close()

        # ---------------- Phase 2: attention, per query panel ----------------
        with ExitStack() as ph:
            maskp = ph.enter_context(tc.tile_pool(name="maskp", bufs=1))
            mask_f = maskp.tile([P, KN], BF16)
            with tc.tile_pool(name="mtmp", bufs=1) as mtmp:
                mask_i = mtmp.tile([P, KN], I32)
                nc.gpsimd.dma_start(out=mask_i[:], in_=m_d.to_broadcast([P, KN]))
                nc.vector.tensor_copy(mask_f[:], mask_i[:])

            e_pool = ph.enter_context(tc.tile_pool(name="em", bufs=2))
            attn_pool = ph.enter_context(tc.tile_pool(name="attnsb", bufs=2))
            at_pool = ph.enter_context(tc.tile_pool(name="attnT", bufs=2))
            ctx_pool = ph.enter_context(tc.tile_pool(name="ctxsb", bufs=2))
            small = ph.enter_context(tc.tile_pool(name="small", bufs=4))
            ps_s = ph.enter_context(tc.tile_pool(name="ps_s", bufs=2, space="PSUM"))
            ps_t = ph.enter_context(tc.tile_pool(name="ps_t", bufs=2, space="PSUM"))
            ps_c = ph.enter_context(tc.tile_pool(name="ps_c", bufs=1, space="PSUM"))

            def scores_half(p, h):
                ps = ps_s.tile([P, 2, 512], F32)
                for dc in range(DC):
                    for n2 in range(2):
                        nc.tensor.matmul(
                            ps[:, n2, :],
                            lhsT=pqt[:, dc, p * P:(p + 1) * P],
                            rhs=pkt[:, dc, (2 * h + n2) * 512:(2 * h + n2 + 1) * 512],
                            start=(dc == 0), stop=(dc == DC - 1))
                return ps

            s_half = [scores_half(0, 0), scores_half(0, 1)]

            for p in range(NPANEL):
                # softmax for panel p (ACT/DVE; overlaps next scores on PE)
                em = e_pool.tile([P, KN], BF16)
                for h in range(2):
                    nc.scalar.activation(
                        em[:, h * 1024:(h + 1) * 1024],
                        s_half[h][:, :, :], AF.Exp,
                        bias=zero_bias[:], scale=SCALE)
                den = small.tile([P, 1], F32)
                nc.vector.scalar_tensor_tensor(
                    out=em[:], in0=em[:], scalar=1.0, in1=mask_f[:],
                    op0=ALU.mult, op1=ALU.mult, accum_out=den[:])
                rden = small.tile([P, 1], F32)
                nc.vector.reciprocal(rden[:], den[:])

                if p + 1 < NPANEL:
                    ns0 = scores_half(p + 1, 0)

                # attn output row-panel (f32)
                attn_sb = attn_pool.tile([P, KN], F32)
                nc.vector.tensor_scalar_mul(attn_sb[:], em[:], rden[:])
                nc.scalar.dma_start(
                    out=attn_d[p * P:(p + 1) * P, :], in_=attn_sb[:])

                # transpose Em (bf16) -> attnT [k_in_s, s, nq]; the 1/den
                # normalization is folded into the ctx copyback scale
                attnT = at_pool.tile([P, NS, P], BF16)
                for g in range(2):
                    pt = ps_t.tile([P, 8, P], BF16)
                    for j in range(8):
                        s_idx = g * 8 + j
                        nc.tensor.transpose(
                            pt[:, j, :],
                            em[:, s_idx * P:(s_idx + 1) * P], ident_bf[:])
                    copyback(attnT[:, g * 8:(g + 1) * 8, :], pt[:, :, :])

                # ctx panel
                ps_ctx = ps_c.tile([P, 2, 512], F32)
                for s in range(NS):
                    for d2 in range(2):
                        nc.tensor.matmul(
                            ps_ctx[:, d2, :],
                            lhsT=attnT[:, s, :],
                            rhs=pv[:, s, d2 * 512:(d2 + 1) * 512],
                            start=(s == 0), stop=(s == NS - 1))
                ctx_sb = ctx_pool.tile([P, D], F32)
                nc.scalar.activation(ctx_sb[:], ps_ctx[:, :, :], AF.Copy,
                                     scale=rden[:])
                nc.scalar.dma_start(
                    out=ctx_d[p * P:(p + 1) * P, :], in_=ctx_sb[:])

                if p + 1 < NPANEL:
                    ns1 = scores_half(p + 1, 1)
                    s_half = [ns0, ns1]

    nc.compile()
    return nc


def _get_nc():
    global _cached_nc
    if _cached_nc is None:
        _cached_nc = _build()
    return _cached_nc


last_exec_time_ns = None


def kernel(**inputs):
    global last_exec_time_ns
    nc = _get_nc()
    query = np.asarray(inputs["query"], dtype=np.float32)
    key = np.asarray(inputs["key"], dtype=np.float32)
    value = np.asarray(inputs["value"], dtype=np.float32)
    mask = np.asarray(inputs["mask"], dtype=np.int32)
    Wq = np.ascontiguousarray(np.asarray(inputs["Wq"], dtype=np.float32))
    bq = np.asarray(inputs["bq"], dtype=np.float32).reshape(1, D)
    Wk = np.ascontiguousarray(np.asarray(inputs["Wk"], dtype=np.float32))
    bk = np.asarray(inputs["bk"], dtype=np.float32).reshape(1, D)
    Wv = np.ascontiguousarray(np.asarray(inputs["Wv"], dtype=np.float32))
    bv = np.asarray(inputs["bv"], dtype=np.float32).reshape(1, D)

    in_maps = []
    for b in range(B):
        in_maps.append({
            "query": np.ascontiguousarray(query[b]),
            "key": np.ascontiguousarray(key[b]),
            "value": np.ascontiguousarray(value[b]),
            "mask": np.ascontiguousarray(mask[b].reshape(1, KN)),
            "Wq": Wq, "bq": bq, "Wk": Wk, "bk": bk, "Wv": Wv, "bv": bv,
        })

    trace = bool(os.environ.get("ATTN_TRACE"))
    res = None
    last_err = None
    for attempt in range(3):
        try:
            res = run_bass_kernel_spmd(nc, in_maps,
                                       core_ids=list(range(N_CORES)),
                                       trace=trace and attempt == 0)
            break
        except Exception as e:  # transient NRT device errors: retry
            last_err = e
    if res is None:
        raise last_err
    last_exec_time_ns = res.exec_time_ns

    ctx = np.stack([res.results[b]["ctx"] for b in range(B)])
    attn = np.stack([res.results[b]["attn"] for b in range(B)])
    return (ctx, attn)
